# revision 5
# baseline (speedup 1.0000x reference)
"""Binarized 3x3 conv (N=32, C=256, H=W=56, pad=1, stride=1) for 8 TRN2 NeuronCores.

Strategy
--------
- Data-parallel over batch: 4 images per core, weight replicated.
- sign-binarized values (+-1) are exact in fp8e4; products are +-1 and the
  conv accumulation (<= 2304 terms) is exact in fp32 PSUM, so the whole
  computation is bit-exact vs the fp32 reference.
- The 3x3 conv is decomposed into 9 shifted matmuls accumulated in PSUM:
  out[co, h, w] += W[co, ci, dh, dw] * x[ci, h+dh, w+dw].
  The image is stored in SBUF zero-padded to 58x58 (plus 1 guard element on
  each end), so every one of the 9 shifted matmuls is a full-size matmul on
  a contiguous window -- no edge cases, the zero padding contributes 0.
- Contraction K = Cin = 256 = 2x128 runs in one pass with fp8 DoubleRow
  (lhsT/rhs get a [128, 2, F] access pattern; PE does 2 MACs/cell/cycle).
- PSUM row blocks: 8 output rows x 58 = 464 fp32 <= 512 (one bank); 7 blocks
  cover 56 rows; garbage columns w=0 and w=57 are dropped on evacuation.
- Binarize is one ScalarE Sign op per 128-channel plane (bias=-1e-30 maps
  exact 0 -> -1 like the reference; the data has no zeros anyway).
"""

import os

import numpy as np

C = 256
H = W = 56
HP = H + 2                      # padded row width (58)
PLANE = HP * HP                 # 3364
PLANE_PAD = 3376                # plane stride; %16==0 (DoubleRow AP step rule)
GUARD = 1                       # guard element before the padded plane
R = 8                           # output rows per PSUM block
NBLK = 7                        # row blocks (7*8 = 56)
NWIN = R * HP                   # matmul free dim: 464


def build_nc(img_per_core=4, mode="fp8", debug=False, *, xp_bufs=2, xs_bufs=2,
             ost_bufs=3, evac="mix"):
    import concourse.bacc as bacc
    import concourse.mybir as mybir
    from concourse import tile

    f32 = mybir.dt.float32
    cdt = mybir.dt.float8e4 if mode == "fp8" else mybir.dt.bfloat16
    pm = mybir.MatmulPerfMode.DoubleRow if mode == "fp8" else None

    nc = bacc.Bacc("TRN2", target_bir_lowering=False, debug=debug)
    x = nc.dram_tensor("x", [img_per_core, C, H, W], f32, kind="ExternalInput")
    wt = nc.dram_tensor("wt", [128, 2, 9, C], f32, kind="ExternalInput")
    y = nc.dram_tensor("y", [img_per_core, C, H, W], f32, kind="ExternalOutput")

    xf = x[:, :, :, :].rearrange("n c h w -> n c (h w)")
    yf = y[:, :, :, :].rearrange("n c h w -> n c (h w)")

    with tile.TileContext(nc) as tc:
        with tc.tile_pool(name="wp", bufs=1) as wp, \
             tc.tile_pool(name="xsp", bufs=xs_bufs) as xsp, \
             tc.tile_pool(name="xqp", bufs=1) as xqp, \
             tc.tile_pool(name="op", bufs=ost_bufs) as op, \
             tc.tile_pool(name="pp", bufs=8, space="PSUM") as pp:

            # Sign bias: tiny negative so exact-0 inputs binarize to -1 like
            # the reference's (x <= 0 -> -1). Needs a const AP of our own.
            bias_t = wp.tile([128, 1], f32, tag="bias")
            nc.vector.memset(bias_t[:, :], -1e-30)
            neg_eps = bias_t[:, 0:1]

            # --- weights: DMA fp32 [128, 2, 9, 256], binarize to cdt ---
            wstage = wp.tile([128, 2, 9, C], f32, tag="wstage")
            nc.sync.dma_start(wstage[:, :, :, :], wt[:, :, :, :])
            w8 = wp.tile([128, 2, 9, C], cdt, tag="w8")
            nc.scalar.sign(w8[:, :, :, :], wstage[:, :, :, :], bias=neg_eps)

            # --- persistent padded input tiles (manual double buffer); zero
            # the pad cells once: [0,60) = guard+row0+col0(row1),
            # [116 + k*58, +2) = col57(row h)+col0(row h+1) for h=1..55,
            # [3306, 3366) = col57(row56)+row57+guard ---
            xps = []
            for b in range(xp_bufs):
                xpq = xqp.tile([128, 2, PLANE_PAD], cdt, tag=f"xp{b}", name=f"xp{b}")
                xps.append(xpq)
                for i in range(2):
                    plane = xpq[:, i, :]
                    nc.vector.memset(plane[:, 0:60], 0)
                    mid = plane[:, 116:116 + 55 * HP].rearrange(
                        "p (r c) -> p r c", c=HP)
                    nc.vector.memset(mid[:, :, 0:2], 0)
                    nc.vector.memset(plane[:, 3306:3366], 0)

            for img in range(img_per_core):
                xp = xps[img % len(xps)]
                xstage = xsp.tile([128, 2, H * W], f32, tag="xstage")
                for i in range(2):
                    nc.sync.dma_start(xstage[:, i, :], xf[img, i * 128:(i + 1) * 128, :])
                # binarize + scatter into padded layout (rows 1..56, cols 1..56)
                for i in range(2):
                    src = xstage[:, i, :].rearrange("p (h w) -> p h w", w=W)
                    dst = xp[:, i, GUARD + HP + 1: GUARD + HP + 1 + H * HP] \
                        .rearrange("p (h w) -> p h w", w=HP)[:, :, 0:W]
                    nc.scalar.sign(dst, src, bias=neg_eps)

                for j in range(2):
                    ost = op.tile([128, H, W], f32, tag="ost", name=f"ost{img}_{j}")
                    pts = [
                        pp.tile([128, R, HP], f32, tag="pt", name=f"pt{img}_{j}_{r}")
                        for r in range(NBLK)
                    ]
                    for ki in range(9):
                        dh, dw = ki // 3 - 1, ki % 3 - 1
                        if mode == "fp8":
                            lhsT = w8[:, :, ki, j * 128:(j + 1) * 128]
                            for r in range(NBLK):
                                off = GUARD + (R * r + 1 + dh) * HP + dw
                                rhs = xp[:, :, off:off + NWIN]
                                nc.tensor.matmul(
                                    pts[r][:, :, :], lhsT, rhs,
                                    start=(ki == 0), stop=(ki == 8),
                                    perf_mode=pm)
                        else:
                            for i in range(2):
                                lhsT = w8[:, i, ki, j * 128:(j + 1) * 128]
                                for r in range(NBLK):
                                    off = GUARD + (R * r + 1 + dh) * HP + dw
                                    rhs = xp[:, i, off:off + NWIN]
                                    nc.tensor.matmul(
                                        pts[r][:, :, :], lhsT, rhs,
                                        start=(ki == 0 and i == 0),
                                        stop=(ki == 8 and i == 1))
                    # evacuate PSUM -> SBUF (drop garbage cols 0 and 57)
                    for r in range(NBLK):
                        src = pts[r][:, :, 1:1 + W]
                        dst = ost[:, R * r:R * r + R, :]
                        use_act = {"mix": r % 2 == 0, "dve": False,
                                   "act": True}[evac]
                        if use_act:
                            nc.scalar.copy(dst, src)
                        else:
                            nc.vector.tensor_copy(dst, src)
                    nc.sync.dma_start(
                        yf[img, j * 128:(j + 1) * 128, :],
                        ost[:, :, :].rearrange("p h w -> p (h w)"))

    nc.compile()
    return nc


def prep_weight(weight: np.ndarray) -> np.ndarray:
    # [co, ci, kh, kw] -> wt[p, i, k, co] = weight[co, i*128+p, kh, kw]
    w5 = weight.reshape(C, 2, 128, 9)
    return np.ascontiguousarray(np.transpose(w5, (2, 1, 3, 0)))


def run(x, weight, n_cores=8, mode="fp8", trace=False):
    from concourse.bass_utils import run_bass_kernel_spmd

    x = np.ascontiguousarray(np.asarray(x, dtype=np.float32))
    weight = np.ascontiguousarray(np.asarray(weight, dtype=np.float32))
    n = x.shape[0]
    per = n // n_cores
    wt = prep_weight(weight)
    nc = build_nc(img_per_core=per, mode=mode)
    in_maps = [
        {"x": x[c * per:(c + 1) * per], "wt": wt} for c in range(n_cores)
    ]
    res = run_bass_kernel_spmd(
        nc, in_maps, core_ids=list(range(n_cores)), trace=trace)
    y = np.concatenate([r["y"] for r in res.results], axis=0)
    return y, res


def kernel(x, weight):
    y, _ = run(x, weight, mode=os.environ.get("BINCONV_MODE", "fp8"))
    return y


# revision 22
# speedup vs baseline: 34612.7031x; 34612.7031x over previous
"""Binarized 3x3 conv (N=32, C=256, H=W=56, pad=1, stride=1) for 8 TRN2 NeuronCores.

Strategy
--------
- Data-parallel over batch: 4 images per core, weight replicated.
- sign-binarized values (+-1) are exact in fp8e4; products are +-1 and the
  conv accumulation (<= 2304 terms) is exact in fp32 PSUM, so the whole
  computation is bit-exact vs the fp32 reference.
- The 3x3 conv is decomposed into 9 shifted matmuls accumulated in PSUM:
  out[co, h, w] += W[co, ci, dh, dw] * x[ci, h+dh, w+dw].
  The image is stored in SBUF zero-padded to 58x58 (plus 1 guard element on
  each end), so every one of the 9 shifted matmuls is a full-size matmul on
  a contiguous window -- no edge cases, the zero padding contributes 0.
- Contraction K = Cin = 256 = 2x128 runs in one pass with fp8 DoubleRow
  (lhsT/rhs get a [128, 2, F] access pattern; PE does 2 MACs/cell/cycle).
- PSUM row blocks: 8 output rows x 58 = 464 fp32 <= 512 (one bank); 7 blocks
  cover 56 rows; garbage columns w=0 and w=57 are dropped on evacuation.
- Binarize is ScalarE Sign (bias=-1e-30 maps exact 0 -> -1 like the
  reference), chunked into row bands so matmuls start before the whole
  image is converted.
- Outputs are integers in [-2304, 2304]: evacuate PSUM as int16 (exact) and
  DMA half the bytes; the host upcasts back to fp32. Input and output DMA
  are the roofline for this shape, so the int16 store is a real win.
"""

import os

import numpy as np

C = 256
H = W = 56
HP = H + 2                      # padded row width (58)
PLANE = HP * HP                 # 3364
PLANE_PAD = 3376                # plane stride; %16==0 (DoubleRow AP step rule)
GUARD = 1                       # guard element before the padded plane
R = 8                           # output rows per PSUM block
NBLK = 7                        # row blocks (7*8 = 56)
NWIN = R * HP                   # matmul free dim: 464
XCHUNK = 4                      # input DMA / sign chunks per plane (56/4 = 14 rows)


def build_nc(img_per_core=4, mode="fp8", debug=False, *, xp_bufs=2, xs_bufs=2,
             ost_bufs=3, evac="mix", out_i16=True, reps=1, hoist_waits=False,
             pe_chain=True):
    import contextlib

    import concourse.bacc as bacc
    import concourse.mybir as mybir
    from concourse import tile
    from concourse.tile_rust import add_dep_helper

    f32 = mybir.dt.float32
    i16 = mybir.dt.int16
    odt = i16 if out_i16 else f32
    cdt = mybir.dt.float8e4 if mode == "fp8" else mybir.dt.bfloat16
    pm = mybir.MatmulPerfMode.DoubleRow if mode == "fp8" else None

    nc = bacc.Bacc("TRN2", target_bir_lowering=False, debug=debug)
    x = nc.dram_tensor("x", [img_per_core, C, H, W], f32, kind="ExternalInput")
    wt = nc.dram_tensor("wt", [128, 2, 9, C], f32, kind="ExternalInput")
    y = nc.dram_tensor("y", [img_per_core, C, H, W], odt, kind="ExternalOutput")

    xf = x[:, :, :, :].rearrange("n c h w -> n c (h w)")
    yf = y[:, :, :, :].rearrange("n c h w -> n c (h w)")

    rows_per_chunk = H // XCHUNK

    with tile.TileContext(nc) as tc:
        with tc.tile_pool(name="wp", bufs=1) as wp, \
             tc.tile_pool(name="xsp", bufs=xs_bufs) as xsp, \
             tc.tile_pool(name="xqp", bufs=1) as xqp, \
             tc.tile_pool(name="op", bufs=ost_bufs) as op, \
             tc.tile_pool(name="pp", bufs=8, space="PSUM") as pp:

            # Sign bias: tiny negative so exact-0 inputs binarize to -1 like
            # the reference's (x <= 0 -> -1). Needs a const AP of our own.
            bias_t = wp.tile([128, 1], f32, tag="bias")
            nc.vector.memset(bias_t[:, :], -1e-30)
            neg_eps = bias_t[:, 0:1]

            # --- weights: DMA fp32 [128, 2, 9, 256] in 3 k-chunks, binarize
            # each as it lands so the first matmuls don't wait for all 9.
            # Only chunk 0 is emitted ahead of image 0's loads (priority);
            # chunks 1-2 are emitted after them (needed only from ki=3 on).
            wstage = wp.tile([128, 2, 9, C], f32, tag="wstage")
            w8 = wp.tile([128, 2, 9, C], cdt, tag="w8")

            W_CHUNKS = [(0, 1), (1, 5), (5, 9)]

            def load_w_chunk(kc):
                ks = slice(*W_CHUNKS[kc])
                nc.sync.dma_start(wstage[:, :, ks, :], wt[:, :, ks, :])
                nc.scalar.sign(w8[:, :, ks, :], wstage[:, :, ks, :], bias=neg_eps)

            load_w_chunk(0)

            # --- persistent padded input tiles (manual multi-buffer); zero
            # the pad cells once: [0,60) = guard+row0+col0(row1),
            # [116 + k*58, +2) = col57(row h)+col0(row h+1) for h=1..55,
            # [3306, 3366) = col57(row56)+row57+guard ---
            xps = []
            for b in range(xp_bufs):
                xpq = xqp.tile([128, 2, PLANE_PAD], cdt, tag=f"xp{b}", name=f"xp{b}")
                xps.append(xpq)
                for i in range(2):
                    plane = xpq[:, i, :]
                    nc.vector.memset(plane[:, 0:60], 0)
                    mid = plane[:, 116:116 + 55 * HP].rearrange(
                        "p (r c) -> p r c", c=HP)
                    nc.vector.memset(mid[:, :, 0:2], 0)
                    nc.vector.memset(plane[:, 3306:3366], 0)

            # Chain matmuls in emission order (ordering-only deps): the
            # scheduler would otherwise interleave matmuls of concurrent
            # groups on the PE stream, which breaks the runs of identical
            # stationary weights that _dedup_ldweights relies on.
            prev_mm = [None]

            def chain(mm):
                if pe_chain and prev_mm[0] is not None:
                    add_dep_helper(mm.ins, prev_mm[0].ins, sync=False,
                                   reason="pe emission order")
                prev_mm[0] = mm

            # reps>1 is a benchmarking aid: repeat the whole pipeline inside
            # one NEFF via a dynamic loop so wall-clock differences between
            # rep counts measure the per-iteration kernel time.
            loop = tc.For_i(0, reps, 1) if reps > 1 else contextlib.nullcontext()
            with loop:
              for img in range(img_per_core):
                xp = xps[img % len(xps)]
                xstage = xsp.tile([128, 2, H * W], f32, tag="xstage")
                # chunked load + binarize: row band c of plane i is signed as
                # soon as its DMA lands, so the first row-block matmuls can
                # start early.
                for c in range(XCHUNK):
                    rs = c * rows_per_chunk
                    seg = slice(rs * W, (rs + rows_per_chunk) * W)
                    for i in range(2):
                        # image 0 is the cold-start critical path: use both
                        # HWDGE rings (SP + ACT) so its 8 load dispatches
                        # don't serialize on one sequencer.
                        eng = nc.scalar if (img == 0 and i == 1) else nc.sync
                        eng.dma_start(
                            xstage[:, i, seg],
                            xf[img, i * 128:(i + 1) * 128, seg])
                    for i in range(2):
                        src = xstage[:, i, seg].rearrange("p (h w) -> p h w", w=W)
                        base = GUARD + (rs + 1) * HP + 1
                        dst = xp[:, i, base:base + rows_per_chunk * HP] \
                            .rearrange("p (h w) -> p h w", w=HP)[:, :, 0:W]
                        nc.scalar.sign(dst, src, bias=neg_eps)
                if img == 0:
                    load_w_chunk(1)
                    load_w_chunk(2)

                for j in range(2):
                    ost = op.tile([128, H, W], odt, tag="ost", name=f"ost{img}_{j}")
                    pts = [
                        pp.tile([128, R, HP], f32, tag="pt", name=f"pt{img}_{j}_{r}")
                        for r in range(NBLK)
                    ]
                    for ki in range(9):
                        dh, dw = ki // 3 - 1, ki % 3 - 1
                        if mode == "fp8":
                            lhsT = w8[:, :, ki, j * 128:(j + 1) * 128]
                            for r in range(NBLK):
                                off = GUARD + (R * r + 1 + dh) * HP + dw
                                rhs = xp[:, :, off:off + NWIN]
                                chain(nc.tensor.matmul(
                                    pts[r][:, :, :], lhsT, rhs,
                                    start=(ki == 0), stop=(ki == 8),
                                    perf_mode=pm))
                        else:
                            for i in range(2):
                                lhsT = w8[:, i, ki, j * 128:(j + 1) * 128]
                                for r in range(NBLK):
                                    off = GUARD + (R * r + 1 + dh) * HP + dw
                                    rhs = xp[:, i, off:off + NWIN]
                                    chain(nc.tensor.matmul(
                                        pts[r][:, :, :], lhsT, rhs,
                                        start=(ki == 0 and i == 0),
                                        stop=(ki == 8 and i == 1)))
                    # evacuate PSUM -> SBUF (drop garbage cols 0 and 57),
                    # converting to int16 (exact: |out| <= 2304). The last
                    # group splits across DVE+ACT (ACT has no more signs to
                    # run) to shorten the kernel tail.
                    last = (img == img_per_core - 1)
                    for r in range(NBLK):
                        src = pts[r][:, :, 1:1 + W]
                        dst = ost[:, R * r:R * r + R, :]
                        use_act = {"mix": r % 2 == 0, "dve": last and r % 2 == 0,
                                   "act": True}[evac]
                        if use_act:
                            nc.scalar.copy(dst, src)
                        else:
                            nc.vector.tensor_copy(dst, src)
                    # store in quarters so early rows fly while later row
                    # blocks are still evacuating. Issued from the
                    # (otherwise idle) GpSimd SWDGE queue: store DMAs wait on
                    # evac sems, and on the SP queue that wait would
                    # head-of-line block the next image's input loads.
                    # (for the final group nothing queues behind the stores,
                    # so use the two idle HWDGE rings instead of the slower
                    # Q7 SWDGE dispatch)
                    ostf = ost[:, :, :].rearrange("p h w -> p (h w)")
                    qh = (H // 4) * W
                    for q in range(4):
                        if last and j == 1:
                            eng = nc.sync if q % 2 == 0 else nc.scalar
                        else:
                            eng = nc.gpsimd
                        eng.dma_start(
                            yf[img, j * 128:(j + 1) * 128, q * qh:(q + 1) * qh],
                            ostf[:, q * qh:(q + 1) * qh])

    _dedup_ldweights(nc)
    if not hoist_waits:
        # bacc's move_matmul_waits_to_ldweights would hoist every matmul's
        # psum-slot wait onto the (deduped) LDW at the head of its weight
        # run, stalling the whole run until all 7 slots are free. Keep the
        # waits on the matmuls themselves instead.
        nc.move_matmul_waits_to_ldweights = lambda: None
    nc.compile()
    return nc


def _dedup_ldweights(nc):
    """Remove InstLdweights that reload the exact weights already resident.

    The Tile layer emits one LDWEIGHTS per matmul even when consecutive
    matmuls share the stationary operand. With DoubleRow (FWL disabled) a
    256-column LDW costs about as much as the matmul itself, so the k-outer /
    r-inner loop (7 matmuls per distinct weight) would pay ~2x PE time.
    Deleting a duplicate is safe: the PE keeps loaded weights until the next
    LDW, and InstMatmult(ldweights=False) does not self-load. Any sem waits
    that Tile parked on a deleted LDW are merged into the next PE
    instruction (waits may only move later in the stream than the deleted
    LDW, never earlier, which preserves ordering).
    """
    import bass_rust

    n_del = 0
    for blk in nc.main_func.blocks:
        out = []
        last_key = None
        pending = []
        for inst in blk.instructions:
            tn = type(inst).__name__
            if tn == "InstLdweights":
                key = (str(inst.ins[0]), str(inst.perf_mode),
                       str(inst.is_transpose), str(inst.tile_position))
                if key == last_key:
                    si = inst.sync_info
                    if si is not None:
                        ups = list(si.on_update)
                        assert not ups, f"dup LDW {inst.name} carries updates"
                        pending.extend(list(si.on_wait))
                    n_del += 1
                    continue
                last_key = key
            if tn in ("InstLdweights", "InstMatmult") and pending:
                si = inst.sync_info
                waits = list(si.on_wait) if si is not None else []
                ups = list(si.on_update) if si is not None else []
                merged = {}
                for w in waits + pending:
                    k2 = (w.id, getattr(w, "wait_mode", None))
                    prev = merged.get(k2)
                    if prev is None or (w.wait_value or 0) > (prev.wait_value or 0):
                        merged[k2] = w
                inst.sync_info = bass_rust.SyncInfo(
                    on_wait=list(merged.values()), on_update=ups)
                pending = []
            out.append(inst)
        assert not pending, "dangling waits from deleted LDW at block end"
        blk.instructions = out
    return n_del


def prep_weight(weight: np.ndarray) -> np.ndarray:
    # [co, ci, kh, kw] -> wt[p, i, k, co] = weight[co, i*128+p, kh, kw]
    w5 = weight.reshape(C, 2, 128, 9)
    return np.ascontiguousarray(np.transpose(w5, (2, 1, 3, 0)))


def run(x, weight, n_cores=8, mode="fp8", trace=False, out_i16=True, **kw):
    from concourse.bass_utils import run_bass_kernel_spmd

    x = np.ascontiguousarray(np.asarray(x, dtype=np.float32))
    weight = np.ascontiguousarray(np.asarray(weight, dtype=np.float32))
    n = x.shape[0]
    per = n // n_cores
    wt = prep_weight(weight)
    nc = build_nc(img_per_core=per, mode=mode, out_i16=out_i16, **kw)
    in_maps = [
        {"x": x[c * per:(c + 1) * per], "wt": wt} for c in range(n_cores)
    ]
    res = run_bass_kernel_spmd(
        nc, in_maps, core_ids=list(range(n_cores)), trace=trace)
    y = np.concatenate([r["y"] for r in res.results], axis=0)
    if y.dtype != np.float32:
        y = y.astype(np.float32)
    return y, res


def kernel(x, weight):
    y, _ = run(x, weight, mode=os.environ.get("BINCONV_MODE", "fp8"))
    return y
